# revision 1
# baseline (speedup 1.0000x reference)
"""GraphSAGE (3x SAGEConv-mean + BN + LeakyReLU) + AvgPool + MLP head on 8 Trainium2
NeuronCores via Bass/Tile.

Sharding: nodes are partitioned contiguously across the 8 cores (2048 each);
weights are replicated (bf16); BatchNorm statistics and per-graph pooled sums
are all-reduced; per-layer activations are all-gathered (node-major bf16 in
HBM) so each core can gather the source rows of its incident edges.

Neighbor mean aggregation: edges are sorted by destination on the host; the
kernel gathers source rows with dma_gather (128-edge chunks on partitions) and
segment-sums them with one-hot S-matrix matmuls on the TensorEngine,
accumulating in PSUM; 1/deg is applied as a per-partition scale.
"""

import math

import numpy as np
import ml_dtypes

BF = ml_dtypes.bfloat16
C = 8          # cores
P = 128        # partitions
EPS = 1e-5
SLOPE = 0.01


# --------------------------------------------------------------------------
# Host-side preprocessing (index manipulation + dtype casts / layout only)
# --------------------------------------------------------------------------

def _tile_w(W):
    """[Kin, Mout] -> [128, Mout/128, Kin/128, 128] so that
    W_sb[p, ko, k, m] = W[k*128+p, ko*128+m] (lhsT column tiles contiguous)."""
    Ki, Mo = W.shape
    return np.ascontiguousarray(
        W.reshape(Ki // P, P, Mo // P, P).transpose(1, 2, 0, 3)
    ).astype(BF)


def _strip(v, ft):
    """[D] -> [128, D/128] fp32 with [p, t] = v[t*128+p]."""
    return np.ascontiguousarray(v.reshape(ft, P).T).astype(np.float32)


def _preprocess(inputs, G=64):
    h = np.asarray(inputs["h"], np.float32)
    src = np.asarray(inputs["src"], np.int64)
    dst = np.asarray(inputs["dst"], np.int64)
    graph_id = np.asarray(inputs["graph_id"], np.int64)
    N, IN_F = h.shape
    HID = np.asarray(inputs["Ws1"]).shape[1]
    MID = np.asarray(inputs["fc2_w"]).shape[1]
    NCLS = np.asarray(inputs["fc3_w"]).shape[1]
    Nc = N // C
    NG = Nc // P          # dst groups (of 128 nodes) per core
    FT = HID // P
    MT = MID // P

    # --- per-core edge partition, sorted by dst, grouped by 128-node groups
    per_core = []
    gmax = np.ones(NG, np.int64)
    for c in range(C):
        lo = c * Nc
        m = (dst >= lo) & (dst < lo + Nc)
        es = src[m]
        ed = dst[m] - lo
        order = np.argsort(ed, kind="stable")
        es, ed = es[order], ed[order]
        gcnt = np.bincount(ed // P, minlength=NG)
        gmax = np.maximum(gmax, gcnt)
        per_core.append((es, ed, gcnt))
    Kg = [int(x) for x in (gmax + P - 1) // P]   # chunks per group (same all cores)
    K = max(Kg)
    EG = K * P                         # padded edge slots per group
    IDXW = EG // 16

    # --- gather indices + one-hot S matrices + 1/deg per core
    idx_all, S_all, invdeg_all, pmat_all = [], [], [], []
    for c in range(C):
        es, ed, gcnt = per_core[c]
        gstart = np.concatenate([[0], np.cumsum(gcnt)])
        idx16 = np.zeros((16, NG, IDXW), np.int16)
        S = np.zeros((NG, P, K, P), np.float32)
        for g in range(NG):
            seg_s = es[gstart[g]:gstart[g + 1]]
            seg_d = ed[gstart[g]:gstart[g + 1]] - g * P
            n = len(seg_s)
            j = np.arange(n)
            idx16[j % 16, g, j // 16] = seg_s.astype(np.int16)
            S[g, j % P, j // P, seg_d] = 1.0
        idx_all.append(np.tile(idx16, (8, 1, 1)))       # replicate for 8 Q7 cores
        S_all.append(S.astype(BF))

        deg = np.bincount(ed, minlength=Nc).astype(np.float64)
        inv = (1.0 / np.maximum(deg, 1.0)).astype(np.float32)
        invdeg_all.append(np.ascontiguousarray(inv.reshape(NG, P).T))

        gid = graph_id[c * Nc:(c + 1) * Nc]
        pm = np.zeros((Nc, G), np.float32)
        pm[np.arange(Nc), gid] = 1.0
        pmat_all.append(
            np.ascontiguousarray(pm.reshape(NG, P, G).transpose(1, 0, 2)).astype(BF)
        )

    cnt = np.bincount(graph_id, minlength=G).astype(np.float64)
    invcnt = (1.0 / np.maximum(cnt, 1.0)).astype(np.float32)[:, None]

    # --- feature tensors
    h128 = np.zeros((N, P), np.float32)
    h128[:, :IN_F] = h
    h128 = h128.astype(BF)
    hT_all = []
    for c in range(C):
        ht = np.zeros((64, Nc), np.float32)
        ht[:IN_F] = h[c * Nc:(c + 1) * Nc].T
        hT_all.append(ht.astype(BF))

    def pad1(W):          # [IN_F, HID] -> [64, FT, 128]
        Wp = np.zeros((64, HID), np.float32)
        Wp[:IN_F] = W
        return np.ascontiguousarray(Wp.reshape(64, FT, P)).astype(BF)

    shared = {
        "h128": h128,
        "w1s": pad1(np.asarray(inputs["Ws1"], np.float32)),
        "w1n": pad1(np.asarray(inputs["Wn1"], np.float32)),
        "w2s": _tile_w(np.asarray(inputs["Ws2"], np.float32)),
        "w2n": _tile_w(np.asarray(inputs["Wn2"], np.float32)),
        "w3s": _tile_w(np.asarray(inputs["Ws3"], np.float32)),
        "w3n": _tile_w(np.asarray(inputs["Wn3"], np.float32)),
        "wf1": _tile_w(np.asarray(inputs["fc1_w"], np.float32)),
        "wf2": _tile_w(np.asarray(inputs["fc2_w"], np.float32)),
        "wf3": np.ascontiguousarray(
            np.asarray(inputs["fc3_w"], np.float32).reshape(MT, P, NCLS)
            .transpose(1, 0, 2)).astype(BF),
        "bn1g": _strip(np.asarray(inputs["g1"], np.float32), FT),
        "bn1b": _strip(np.asarray(inputs["be1"], np.float32), FT),
        "bn2g": _strip(np.asarray(inputs["g2"], np.float32), FT),
        "bn2b": _strip(np.asarray(inputs["be2"], np.float32), FT),
        "bn3g": _strip(np.asarray(inputs["g3"], np.float32), FT),
        "bn3b": _strip(np.asarray(inputs["be3"], np.float32), FT),
        "f1b": _strip(np.asarray(inputs["fc1_b"], np.float32), FT),
        "f2b": _strip(np.asarray(inputs["fc2_b"], np.float32), MT),
        "f3b": np.asarray(inputs["fc3_b"], np.float32)[:, None].copy(),
        "invcnt": invcnt,
        "chain": np.zeros((G, NCLS), np.float32),
    }
    in_maps = []
    for c in range(C):
        m = dict(shared)
        m.update({
            "hT": hT_all[c],
            "gidx": idx_all[c],
            "smat": S_all[c],
            "invdeg": invdeg_all[c],
            "pmat": pmat_all[c],
        })
        in_maps.append(m)

    meta = dict(N=N, Nc=Nc, NG=NG, FT=FT, MT=MT, HID=HID, MID=MID, NCLS=NCLS,
                K=K, EG=EG, IDXW=IDXW, G=G, Kg=Kg)
    return in_maps, meta


# --------------------------------------------------------------------------
# Bass program
# --------------------------------------------------------------------------

def _build(meta):
    import concourse.bass as bass
    import concourse.mybir as mybir
    import concourse.tile as tile
    from concourse import bacc
    from concourse.masks import make_identity

    dt = mybir.dt
    ALU = mybir.AluOpType
    ACT = mybir.ActivationFunctionType

    N, Nc, NG, FT, MT = meta["N"], meta["Nc"], meta["NG"], meta["FT"], meta["MT"]
    HID, MID, NCLS = meta["HID"], meta["MID"], meta["NCLS"]
    K, EG, IDXW, G = meta["K"], meta["EG"], meta["IDXW"], meta["G"]
    Kg = meta["Kg"]
    CH = min(1024, Nc)                 # dense node-chunk
    NCH = Nc // CH
    HH = CH // 512                     # 512-wide halves per chunk
    NACC = NCH * HH
    NQ = Nc // 512                     # bn-apply quarters
    QF = HID // 512                    # 512-wide feature quarters

    import os
    NOCC = bool(os.environ.get("GCN_NOCC"))
    rg = [list(range(C))]

    nc = bacc.Bacc("TRN2", target_bir_lowering=False, debug=False,
                   num_devices=1 if NOCC else C)

    def collective(kind, op, ins, outs):
        if NOCC:
            iap, oap = ins[0], outs[0]
            if kind == "AllGather":
                nc.sync.dma_start(oap[:iap.shape[0]], iap)
            else:
                nc.sync.dma_start(oap, iap)
        else:
            nc.gpsimd.collective_compute(kind, op, replica_groups=rg,
                                         ins=[ins[0].opt()], outs=[outs[0].opt()])

    # ---- inputs
    t_h128 = nc.dram_tensor("h128", [N, P], dt.bfloat16, kind="ExternalInput")
    t_hT = nc.dram_tensor("hT", [64, Nc], dt.bfloat16, kind="ExternalInput")
    t_gidx = nc.dram_tensor("gidx", [P, NG, IDXW], dt.int16, kind="ExternalInput")
    t_smat = nc.dram_tensor("smat", [NG, P, K, P], dt.bfloat16, kind="ExternalInput")
    t_invdeg = nc.dram_tensor("invdeg", [P, NG], dt.float32, kind="ExternalInput")
    t_w1s = nc.dram_tensor("w1s", [64, FT, P], dt.bfloat16, kind="ExternalInput")
    t_w1n = nc.dram_tensor("w1n", [64, FT, P], dt.bfloat16, kind="ExternalInput")
    t_w = {}
    for nm in ("w2s", "w2n", "w3s", "w3n", "wf1"):
        t_w[nm] = nc.dram_tensor(nm, [P, FT, FT, P], dt.bfloat16, kind="ExternalInput")
    t_w["wf2"] = nc.dram_tensor("wf2", [P, MT, FT, P], dt.bfloat16, kind="ExternalInput")
    t_wf3 = nc.dram_tensor("wf3", [P, MT, NCLS], dt.bfloat16, kind="ExternalInput")
    t_bn = {}
    for nm in ("bn1g", "bn1b", "bn2g", "bn2b", "bn3g", "bn3b", "f1b"):
        t_bn[nm] = nc.dram_tensor(nm, [P, FT], dt.float32, kind="ExternalInput")
    t_bn["f2b"] = nc.dram_tensor("f2b", [P, MT], dt.float32, kind="ExternalInput")
    t_f3b = nc.dram_tensor("f3b", [NCLS, 1], dt.float32, kind="ExternalInput")
    t_pmat = nc.dram_tensor("pmat", [P, NG, G], dt.bfloat16, kind="ExternalInput")
    t_invcnt = nc.dram_tensor("invcnt", [G, 1], dt.float32, kind="ExternalInput")
    t_out = nc.dram_tensor("out", [G, NCLS], dt.float32, kind="ExternalOutput")
    t_chain = nc.dram_tensor("chain", [G, NCLS], dt.float32, kind="ExternalInput")

    with tile.TileContext(nc) as tc:
        import contextlib
        ctx = contextlib.ExitStack()
        with ctx:
            dram = ctx.enter_context(tc.tile_pool(name="dram", bufs=1, space="DRAM"))
            consts = ctx.enter_context(tc.tile_pool(name="consts", bufs=1))
            work = ctx.enter_context(tc.tile_pool(name="work", bufs=1))
            psp = ctx.enter_context(tc.tile_pool(name="psp", bufs=8, space="PSUM"))

            # ---- DRAM scratch
            rst_hbm = dram.tile([P, FT, Nc], dt.bfloat16)
            m_hbm = dram.tile([P, FT, Nc], dt.bfloat16)
            sp_hbm = dram.tile([P, FT, Nc], dt.bfloat16)
            ynm = dram.tile([Nc, HID], dt.bfloat16)
            yfull = [dram.tile([N, HID], dt.bfloat16, addr_space="Shared",
                               name=f"yfull{i}") for i in range(2)]
            stat_in = [dram.tile([P, 2 * FT], dt.float32, name=f"sti{i}")
                       for i in range(3)]
            stat_out = [dram.tile([P, 2 * FT], dt.float32, addr_space="Shared",
                                  name=f"sto{i}") for i in range(3)]
            pool_in = dram.tile([G, HID], dt.float32)
            pool_out = dram.tile([G, HID], dt.float32, addr_space="Shared")

            # ---- constants to SBUF
            idx_t = consts.tile([P, NG, IDXW], dt.int16)
            nc.sync.dma_start(idx_t[:], t_gidx[:])
            invdeg_t = consts.tile([P, NG], dt.float32)
            nc.sync.dma_start(invdeg_t[:], t_invdeg[:])
            hT_t = consts.tile([64, Nc], dt.bfloat16)
            nc.sync.dma_start(hT_t[:], t_hT[:])
            w1s_t = consts.tile([64, FT, P], dt.bfloat16)
            nc.sync.dma_start(w1s_t[:], t_w1s[:])
            w1n_t = consts.tile([64, FT, P], dt.bfloat16)
            nc.sync.dma_start(w1n_t[:], t_w1n[:])
            pmat_t = consts.tile([P, NG, G], dt.bfloat16)
            nc.sync.dma_start(pmat_t[:], t_pmat[:])
            invcnt_t = consts.tile([G, 1], dt.float32)
            nc.sync.dma_start(invcnt_t[:], t_invcnt[:])
            wf3_t = consts.tile([P, MT, NCLS], dt.bfloat16)
            nc.sync.dma_start(wf3_t[:], t_wf3[:])
            f3b_t = consts.tile([NCLS, 1], dt.float32)
            nc.sync.dma_start(f3b_t[:], t_f3b[:])
            bn_t = {}
            for nm, th in t_bn.items():
                bn_t[nm] = consts.tile(list(th.shape), dt.float32, name=f"c_{nm}")
                nc.sync.dma_start(bn_t[nm][:], th[:])
            ident_bf = consts.tile([P, P], dt.bfloat16)
            make_identity(nc, ident_bf[:])
            ident_f32 = consts.tile([P, P], dt.float32)
            make_identity(nc, ident_f32[:])
            m0_fm = consts.tile([64, Nc], dt.bfloat16)

            # ---------------- helpers ----------------
            def agg_phase(li):
                """neighbor mean into m (layer1 -> m0_fm sbuf, else m_hbm)."""
                ew = P if li == 1 else HID          # gathered row width
                gsrc = t_h128 if li == 1 else yfull[li - 2]
                for g in range(NG):
                    KG = Kg[g]
                    S_g = work.tile([P, KG, P], dt.bfloat16, tag="sg", bufs=2,
                                    name=f"sg{li}_{g}")
                    nc.sync.dma_start(S_g[:], t_smat[g, :, :KG, :])
                    Gt = work.tile([P, KG, ew], dt.bfloat16, tag="big", bufs=3,
                                   name=f"G{li}_{g}")
                    nc.gpsimd.dma_gather(
                        out_ap=Gt[:],
                        in_ap=gsrc[:],
                        idxs_ap=idx_t[:, g, :KG * 8],
                        num_idxs=KG * P,
                        num_idxs_reg=KG * P,
                        elem_size=ew,
                    )
                    if li == 1:
                        ps = psp.tile([P, 512], dt.float32, tag="ps",
                                      name=f"aps{li}_{g}")
                        for k in range(KG):
                            nc.tensor.matmul(ps[:, :P], lhsT=S_g[:, k, :],
                                             rhs=Gt[:, k, :P],
                                             start=(k == 0), stop=(k == KG - 1))
                        mnm = work.tile([P, P], dt.bfloat16, tag="mnm0", bufs=2,
                                        name=f"mnm{li}_{g}")
                        nc.vector.tensor_scalar(mnm[:], ps[:, :P],
                                                invdeg_t[:, g:g + 1], None, ALU.mult)
                        tp = psp.tile([P, 256], dt.bfloat16, tag="ps",
                                      name=f"tp{li}_{g}")
                        nc.tensor.transpose(tp[:64, :P], mnm[:, :64], ident_bf[:])
                        nc.vector.tensor_copy(m0_fm[:, g * P:(g + 1) * P],
                                              tp[:64, :P])
                    else:
                        mnm = work.tile([P, HID], dt.bfloat16, tag="mnm", bufs=2,
                                        name=f"mnm{li}_{g}")
                        for q in range(QF):
                            ps = psp.tile([P, 512], dt.float32, tag="ps",
                                          name=f"aps{li}_{g}_{q}")
                            for k in range(KG):
                                nc.tensor.matmul(ps[:], lhsT=S_g[:, k, :],
                                                 rhs=Gt[:, k, q * 512:(q + 1) * 512],
                                                 start=(k == 0), stop=(k == KG - 1))
                            nc.vector.tensor_scalar(mnm[:, q * 512:(q + 1) * 512],
                                                    ps[:], invdeg_t[:, g:g + 1],
                                                    None, ALU.mult)
                        mTg = work.tile([P, FT, P], dt.bfloat16, tag="mTg", bufs=2,
                                        name=f"mTg{li}_{g}")
                        for ft in range(FT):
                            tp = psp.tile([P, 256], dt.bfloat16, tag="ps",
                                          name=f"tp{li}_{g}_{ft}")
                            nc.tensor.transpose(tp[:, :P],
                                                mnm[:, ft * P:(ft + 1) * P],
                                                ident_bf[:])
                            if ft % 2 == 0:
                                nc.vector.tensor_copy(mTg[:, ft, :], tp[:, :P])
                            else:
                                nc.scalar.copy(mTg[:, ft, :], tp[:, :P])
                        nc.sync.dma_start(m_hbm[:, :, g * P:(g + 1) * P], mTg[:])

            def pass_a(li, ch):
                """self-path matmuls of layer li (>=2) for node chunk ch,
                reading y(li-1) from ynm; bf16 partials -> sp_hbm."""
                n0 = ch * CH
                y_ch = work.tile([P, FT, CH], dt.bfloat16, tag="big", bufs=3,
                                 name=f"ya{li}_{ch}")
                for kt in range(FT):
                    nc.sync.dma_start_transpose(
                        y_ch[:, kt, :], ynm[n0:n0 + CH, kt * P:(kt + 1) * P])
                ws_d = t_w[f"w{li}s"]
                for fo in range(FT):
                    wsc = work.tile([P, FT, P], dt.bfloat16, tag="wcol",
                                    bufs=6, name=f"wa{li}_{ch}_{fo}")
                    nc.sync.dma_start(wsc[:], ws_d[:, fo])
                    for hh in range(HH):
                        ps = psp.tile([P, 512], dt.float32, tag="ps",
                                      name=f"pa{li}_{ch}_{fo}_{hh}")
                        sl = slice(hh * 512, hh * 512 + 512)
                        for k in range(FT):
                            nc.tensor.matmul(ps[:], lhsT=wsc[:, k, :],
                                             rhs=y_ch[:, k, sl],
                                             start=(k == 0), stop=(k == FT - 1))
                        sp = work.tile([P, 512], dt.bfloat16, tag="spt",
                                       bufs=3, name=f"sp{li}_{ch}_{fo}_{hh}")
                        nc.scalar.copy(sp[:], ps[:])
                        nc.sync.dma_start(
                            sp_hbm[:, fo, n0 + hh * 512:n0 + hh * 512 + 512],
                            sp[:])

            def dense_phase(li, ssum, ssq):
                """rst = W_s.T @ y + W_n.T @ m (+stats) -> rst_hbm (bf16)."""
                for ch in range(NCH):
                    n0 = ch * CH
                    if li > 1:
                        sp_ch = work.tile([P, FT, CH], dt.bfloat16, tag="big",
                                          bufs=3, name=f"sch{li}_{ch}")
                        nc.sync.dma_start(sp_ch[:], sp_hbm[:, :, n0:n0 + CH])
                        m_ch = work.tile([P, FT, CH], dt.bfloat16, tag="big",
                                         bufs=3, name=f"mch{li}_{ch}")
                        nc.sync.dma_start(m_ch[:], m_hbm[:, :, n0:n0 + CH])
                    wn_d = t_w[f"w{li}n"] if li > 1 else None
                    for fo in range(FT):
                        if li > 1:
                            wnc = work.tile([P, FT, P], dt.bfloat16, tag="wcol",
                                            bufs=6, name=f"wnc{li}_{ch}_{fo}")
                            nc.sync.dma_start(wnc[:], wn_d[:, fo])
                        for hh in range(HH):
                            ps = psp.tile([P, 512], dt.float32, tag="ps",
                                          name=f"dps{li}_{ch}_{fo}_{hh}")
                            sl = slice(hh * 512, hh * 512 + 512)
                            gl = slice(n0 + hh * 512, n0 + hh * 512 + 512)
                            if li == 1:
                                nc.tensor.matmul(ps[:], lhsT=w1s_t[:, fo, :],
                                                 rhs=hT_t[:, gl], start=True,
                                                 stop=False)
                                nc.tensor.matmul(ps[:], lhsT=w1n_t[:, fo, :],
                                                 rhs=m0_fm[:, gl], start=False,
                                                 stop=True)
                            else:
                                nc.tensor.matmul(ps[:], lhsT=ident_bf[:],
                                                 rhs=sp_ch[:, fo, sl],
                                                 start=True, stop=False)
                                for k in range(FT):
                                    nc.tensor.matmul(ps[:], lhsT=wnc[:, k, :],
                                                     rhs=m_ch[:, k, sl],
                                                     start=False,
                                                     stop=(k == FT - 1))
                            rstt = work.tile([P, 512], dt.bfloat16, tag="rstt",
                                             bufs=3, name=f"rt{li}_{ch}_{fo}_{hh}")
                            nc.scalar.activation(
                                rstt[:], ps[:], ACT.Copy,
                                accum_out=ssum[:, fo, ch * HH + hh:ch * HH + hh + 1])
                            junk = work.tile([P, 512], dt.bfloat16, tag="junk",
                                             bufs=2, name=f"jk{li}_{ch}_{fo}_{hh}")
                            nc.scalar.activation(
                                junk[:], rstt[:], ACT.Square,
                                accum_out=ssq[:, fo, ch * HH + hh:ch * HH + hh + 1])
                            nc.sync.dma_start(
                                rst_hbm[:, fo, n0 + hh * 512:n0 + hh * 512 + 512],
                                rstt[:])

            def stats_phase(li, ssum, ssq):
                """AllReduce sums -> a, b affine coefficients."""
                acc_s = work.tile([P, FT], dt.float32, tag="acc", bufs=4,
                                  name=f"as{li}")
                acc_q = work.tile([P, FT], dt.float32, tag="acc", bufs=4,
                                  name=f"aq{li}")
                nc.vector.tensor_copy(acc_s[:], ssum[:, :, 0])
                nc.vector.tensor_copy(acc_q[:], ssq[:, :, 0])
                for j in range(1, NACC):
                    nc.vector.tensor_tensor(acc_s[:], acc_s[:], ssum[:, :, j],
                                            ALU.add)
                    nc.vector.tensor_tensor(acc_q[:], acc_q[:], ssq[:, :, j],
                                            ALU.add)
                nc.sync.dma_start(stat_in[li - 1][:, :FT], acc_s[:])
                nc.sync.dma_start(stat_in[li - 1][:, FT:], acc_q[:])
                collective("AllReduce", ALU.add, [stat_in[li - 1]],
                           [stat_out[li - 1]])
                sums = work.tile([P, 2 * FT], dt.float32, tag="sums", bufs=2,
                                 name=f"sm{li}")
                nc.sync.dma_start(sums[:], stat_out[li - 1][:])
                mu = work.tile([P, FT], dt.float32, tag="acc", bufs=4,
                               name=f"mu{li}")
                var = work.tile([P, FT], dt.float32, tag="acc", bufs=4,
                                name=f"vr{li}")
                nc.vector.tensor_scalar(mu[:], sums[:, :FT], 1.0 / N, None,
                                        ALU.mult)
                nc.vector.tensor_scalar(var[:], sums[:, FT:], 1.0 / N, None,
                                        ALU.mult)
                tmp = work.tile([P, FT], dt.float32, tag="acc2", bufs=4,
                                name=f"tm{li}")
                nc.vector.tensor_tensor(tmp[:], mu[:], mu[:], ALU.mult)
                nc.vector.tensor_tensor(var[:], var[:], tmp[:], ALU.subtract)
                nc.vector.tensor_scalar(var[:], var[:], EPS, None, ALU.add)
                std = work.tile([P, FT], dt.float32, tag="acc2", bufs=4,
                                name=f"sd{li}")
                nc.scalar.activation(std[:], var[:], ACT.Sqrt)
                rstd = work.tile([P, FT], dt.float32, tag="acc2", bufs=4,
                                 name=f"rs{li}")
                nc.vector.reciprocal(rstd[:], std[:])
                a_sb = work.tile([P, FT], dt.float32, tag="ab", bufs=2,
                                 name=f"a{li}")
                b_sb = work.tile([P, FT], dt.float32, tag="ab", bufs=2,
                                 name=f"b{li}")
                nc.vector.tensor_tensor(a_sb[:], rstd[:], bn_t[f"bn{li}g"][:],
                                        ALU.mult)
                nc.vector.tensor_tensor(tmp[:], mu[:], a_sb[:], ALU.mult)
                nc.vector.tensor_tensor(b_sb[:], bn_t[f"bn{li}b"][:], tmp[:],
                                        ALU.subtract)
                return a_sb, b_sb

            def bn_apply_phase(li, a_sb, b_sb, pool_ps):
                do_pass_a = li < 3
                """y = lrelu(a*rst+b); transpose; -> ynm (li<3) / pooling (li=3)."""
                for qq in range(NQ):
                    q0 = qq * 512
                    yT = [work.tile([P, HID], dt.bfloat16, tag="yT", bufs=4,
                                    name=f"yT{li}_{qq}_{j}") for j in range(4)]
                    for ft in range(FT):
                        rstf = work.tile([P, 512], dt.bfloat16, tag="rstf",
                                         bufs=2, name=f"rf{li}_{qq}_{ft}")
                        nc.sync.dma_start(rstf[:], rst_hbm[:, ft, q0:q0 + 512])
                        z = work.tile([P, 512], dt.float32, tag="z", bufs=2,
                                      name=f"z{li}_{qq}_{ft}")
                        nc.vector.tensor_scalar(z[:], rstf[:],
                                                a_sb[:, ft:ft + 1],
                                                b_sb[:, ft:ft + 1],
                                                ALU.mult, ALU.add)
                        z01 = work.tile([P, 512], dt.float32, tag="z01", bufs=2,
                                        name=f"zs{li}_{qq}_{ft}")
                        nc.scalar.mul(z01[:], z[:], SLOPE)
                        yq = work.tile([P, 512], dt.bfloat16, tag="yq", bufs=2,
                                       name=f"yq{li}_{qq}_{ft}")
                        nc.vector.tensor_tensor(yq[:], z[:], z01[:], ALU.max)
                        for j in range(4):
                            tp = psp.tile([P, 256], dt.bfloat16, tag="ps",
                                          name=f"ytp{li}_{qq}_{ft}_{j}")
                            nc.tensor.transpose(tp[:, :P],
                                                yq[:, j * P:(j + 1) * P],
                                                ident_bf[:])
                            if j % 2 == 0:
                                nc.vector.tensor_copy(
                                    yT[j][:, ft * P:(ft + 1) * P], tp[:, :P])
                            else:
                                nc.scalar.copy(
                                    yT[j][:, ft * P:(ft + 1) * P], tp[:, :P])
                    for j in range(4):
                        nt = qq * 4 + j
                        if li < 3:
                            nc.sync.dma_start(ynm[nt * P:(nt + 1) * P, :], yT[j])
                        else:
                            for q in range(QF):
                                nc.tensor.matmul(
                                    pool_ps[q][:G],
                                    lhsT=pmat_t[:, nt, :],
                                    rhs=yT[j][:, q * 512:(q + 1) * 512],
                                    start=(nt == 0), stop=(nt == NG - 1),
                                    skip_group_check=True)
                    if do_pass_a and ((qq + 1) * 512) % CH == 0:
                        pass_a(li + 1, ((qq + 1) * 512) // CH - 1)

            # ---------------- the network ----------------
            import os
            STAGE = os.environ.get("GCN_STAGE", "full")
            for li in (1, 2, 3):
                ssum = work.tile([P, FT, NACC], dt.float32, tag="stats", bufs=2,
                                 name=f"ssum{li}")
                ssq = work.tile([P, FT, NACC], dt.float32, tag="stats", bufs=2,
                                name=f"ssq{li}")
                agg_phase(li)
                if STAGE == f"agg{li}":
                    nc.gpsimd.dma_start(t_out[:], m0_fm[:G, :NCLS])
                    break
                dense_phase(li, ssum, ssq)
                if STAGE == f"dense{li}":
                    nc.gpsimd.dma_start(t_out[:], m0_fm[:G, :NCLS])
                    break
                a_sb, b_sb = stats_phase(li, ssum, ssq)
                if STAGE == f"stats{li}":
                    nc.gpsimd.dma_start(t_out[:], m0_fm[:G, :NCLS])
                    break
                if li == 3:
                    pool_ps = [psp.tile([P, 512], dt.float32, tag="ps",
                                        name=f"pps{q}") for q in range(QF)]
                else:
                    pool_ps = None
                bn_apply_phase(li, a_sb, b_sb, pool_ps)
                if STAGE == f"bn{li}":
                    nc.gpsimd.dma_start(t_out[:], m0_fm[:G, :NCLS])
                    break
                if li < 3:
                    collective("AllGather", ALU.bypass, [ynm], [yfull[li - 1]])
                if STAGE == f"ag{li}":
                    nc.gpsimd.dma_start(t_out[:], m0_fm[:G, :NCLS])
                    break

            if STAGE == "full":
                # ---------------- pooling + head ----------------
                hgsb = work.tile([G, HID], dt.float32, tag="big", bufs=3, name="hgsb")
                for q in range(QF):
                    nc.vector.tensor_copy(hgsb[:, q * 512:(q + 1) * 512],
                                          pool_ps[q][:G])
                nc.sync.dma_start(pool_in[:], hgsb[:])
                collective("AllReduce", ALU.add, [pool_in], [pool_out])
                hgr0 = work.tile([G, HID], dt.float32, tag="big", bufs=3, name="hgr0")
                nc.sync.dma_start(hgr0[:], pool_out[:])
                hgr = work.tile([G, HID], dt.float32, tag="big", bufs=3, name="hgr")
                nc.vector.tensor_scalar(hgr[:], hgr0[:], invcnt_t[:, 0:1], None,
                                        ALU.mult)
                hg_fm = work.tile([P, FT, G], dt.bfloat16, tag="hgfm", bufs=1)
                for ft in range(FT):
                    tp = psp.tile([P, 256], dt.bfloat16, tag="ps", name=f"htp{ft}")
                    tpf = tp.bitcast(dt.float32)
                    nc.tensor.transpose(tpf[:, :G], hgr[:, ft * P:(ft + 1) * P],
                                        ident_f32[:G, :G])
                    nc.vector.tensor_copy(hg_fm[:, ft, :], tpf[:, :G])

                def fc_layer(win, kt_count, fo_count, xin, bias_t, name):
                    xout = work.tile([P, fo_count, G], dt.bfloat16, tag=f"x{name}",
                                     bufs=1, name=f"x{name}")
                    for fo in range(fo_count):
                        wc = work.tile([P, kt_count, P], dt.bfloat16, tag="wcol",
                                       bufs=6, name=f"w{name}_{fo}")
                        nc.sync.dma_start(wc[:], win[:, fo])
                        ps = psp.tile([P, 512], dt.float32, tag="ps",
                                      name=f"hps{name}_{fo}")
                        for k in range(kt_count):
                            nc.tensor.matmul(ps[:, :G], lhsT=wc[:, k, :],
                                             rhs=xin[:, k, :],
                                             start=(k == 0), stop=(k == kt_count - 1))
                        zh = work.tile([P, G], dt.float32, tag="zh", bufs=2,
                                       name=f"zh{name}_{fo}")
                        nc.vector.tensor_scalar(zh[:], ps[:, :G],
                                                bias_t[:, fo:fo + 1], None, ALU.add)
                        zh2 = work.tile([P, G], dt.float32, tag="zh2", bufs=2,
                                        name=f"z2{name}_{fo}")
                        nc.scalar.mul(zh2[:], zh[:], SLOPE)
                        nc.vector.tensor_tensor(xout[:, fo, :], zh[:], zh2[:],
                                                ALU.max)
                    return xout

                x1 = fc_layer(t_w["wf1"], FT, FT, hg_fm, bn_t["f1b"], "f1")
                x2 = fc_layer(t_w["wf2"], FT, MT, x1, bn_t["f2b"], "f2")

                ps18 = psp.tile([P, 512], dt.float32, tag="ps", name="ps18")
                for k in range(MT):
                    nc.tensor.matmul(ps18[:NCLS, :G], lhsT=wf3_t[:, k, :],
                                     rhs=x2[:, k, :], start=(k == 0),
                                     stop=(k == MT - 1))
                o18 = work.tile([NCLS, G], dt.float32, tag="o18", bufs=1)
                nc.vector.tensor_scalar(o18[:], ps18[:NCLS, :G], f3b_t[:, 0:1],
                                        None, ALU.add)
                tp = psp.tile([P, 256], dt.bfloat16, tag="ps", name="otp")
                tpf = tp.bitcast(dt.float32)
                nc.tensor.transpose(tpf[:G, :NCLS], o18[:], ident_f32[:NCLS, :NCLS])
                osb = work.tile([G, NCLS], dt.float32, tag="osb", bufs=1)
                nc.vector.tensor_copy(osb[:], tpf[:G, :NCLS])
                chn = work.tile([G, NCLS], dt.float32, tag="chn", bufs=1)
                nc.sync.dma_start(chn[:], t_chain[:])
                nc.vector.tensor_scalar(chn[:], chn[:], 0.0, None, ALU.mult)
                nc.vector.tensor_tensor(osb[:], osb[:], chn[:], ALU.add)
                nc.sync.dma_start(t_out[:], osb[:])


    nc.compile()
    return nc


# --------------------------------------------------------------------------
# entry point
# --------------------------------------------------------------------------

LAST_EXEC_NS = None
LAST_TRACE = None


def _run_timed(nc, in_maps, iters=4, reps=None):
    """Mirror bass2jax.run_bass_via_pjrt but keep inputs device-resident so
    warm re-executions measure the on-device program span."""
    import time
    import jax
    import jax.numpy as jnp
    from jax.sharding import Mesh, PartitionSpec
    from jax.experimental.shard_map import shard_map
    import concourse.mybir as mybir
    from concourse.bass2jax import (
        install_neuronx_cc_hook, _bass_exec_p, partition_id_tensor)

    install_neuronx_cc_hook()
    n_cores = len(in_maps)
    partition_name = nc.partition_id_tensor.name if nc.partition_id_tensor else None
    in_names, out_names, out_avals, zero_outs = [], [], [], []
    for alloc in nc.m.functions[0].allocations:
        if not isinstance(alloc, mybir.MemoryLocationSet):
            continue
        name = alloc.memorylocations[0].name
        if alloc.kind == "ExternalInput":
            if name != partition_name:
                in_names.append(name)
        elif alloc.kind == "ExternalOutput":
            shape = tuple(alloc.tensor_shape)
            dtype = mybir.dt.np(alloc.dtype)
            out_names.append(name)
            out_avals.append(jax.core.ShapedArray(shape, dtype))
            zero_outs.append(np.zeros((n_cores * shape[0], *shape[1:]), dtype))
    n_params = len(in_names)
    all_in = list(in_names) + list(out_names)
    if partition_name is not None:
        all_in.append(partition_name)

    import os
    if reps is None:
        reps = int(os.environ.get("GCN_REPS", "1"))

    chain_idx = in_names.index("chain") if "chain" in in_names else None
    out_idx = out_names.index("out") if "out" in out_names else None

    def _body(*args):
        operands = list(args)
        if partition_name is not None:
            operands.append(partition_id_tensor())
        for _ in range(reps):
            outs = _bass_exec_p.bind(
                *operands, out_avals=tuple(out_avals), in_names=tuple(all_in),
                out_names=tuple(out_names), lowering_input_output_aliases=(),
                sim_require_finite=True, sim_require_nnan=True, nc=nc)
            if chain_idx is not None and out_idx is not None:
                operands[chain_idx] = outs[out_idx]
        return tuple(outs)

    devices = jax.devices()[:n_cores]
    mesh = Mesh(np.asarray(devices), ("core",))
    nin = n_params + len(out_names)
    sharded = jax.jit(
        shard_map(_body, mesh=mesh, in_specs=(PartitionSpec("core"),) * nin,
                  out_specs=(PartitionSpec("core"),) * len(out_names),
                  check_rep=False),
        donate_argnums=tuple(range(n_params, nin)), keep_unused=True)

    shd = jax.sharding.NamedSharding(mesh, PartitionSpec("core"))
    dev_in = [
        jax.device_put(
            np.concatenate([np.asarray(in_maps[c][nm]) for c in range(n_cores)],
                           axis=0), shd)
        for nm in in_names
    ]
    times = []
    outs = None
    for _ in range(iters):
        zo = [jax.device_put(z.copy(), shd) for z in zero_outs]
        for z in zo:
            z.block_until_ready()
        t0 = time.perf_counter()
        outs = sharded(*dev_in, *zo)
        for o in outs:
            o.block_until_ready()
        times.append(time.perf_counter() - t0)
    best_ns = int(min(times) * 1e9 / reps)
    results = [
        {nm: np.asarray(outs[i]).reshape(n_cores, *out_avals[i].shape)[c]
         for i, nm in enumerate(out_names)}
        for c in range(n_cores)
    ]
    print(f"timed runs (s, reps={reps}): {[f'{t:.4f}' for t in times]}")
    return results, best_ns


def kernel(**inputs) -> np.ndarray:
    global LAST_EXEC_NS, LAST_TRACE
    from concourse.bass_utils import run_bass_kernel_spmd

    import os

    in_maps, meta = _preprocess(inputs)
    nc = _build(meta)
    in_maps = [{k: np.ascontiguousarray(v) for k, v in m.items()}
               for m in in_maps]
    if os.environ.get("GCN_TIME"):
        results, best_ns = _run_timed(nc, in_maps)
        LAST_EXEC_NS = best_ns
        return np.asarray(results[0]["out"], np.float32)
    res = run_bass_kernel_spmd(nc, in_maps, core_ids=list(range(C)))
    LAST_EXEC_NS = res.exec_time_ns
    LAST_TRACE = res.instructions_and_trace
    return np.asarray(res.results[0]["out"], np.float32)

